# revision 18
# baseline (speedup 1.0000x reference)
"""Causal multi-head attention (B=4, T=2048, C=1024, 16 heads) on 8 TRN2 cores.

Tensor-parallel over heads: core c owns heads 2c, 2c+1 (128 features) and
computes Q/K/V projections + attention for those heads over ALL 4 batches.
Causal work uses exact extents (q-chunk 512, kv extent 512*(qc+1)); for the
four diagonal kv blocks only the columns right of the diagonal are computed
(W = 512-128r), so just one 128x128 tril strip needs masking. Every core runs
the same instruction stream (SPMD) - only weight slices differ.

After attention, a per-batch 512KB AllToAll redistributes context from
feature-sharded to token-sharded; each core then runs the output projection
for its 256-token slice of each batch (contract over the full 1024 features).
Output projections are deferred (batch 0's into batch 3's attention, batches
1-2's to the tail) so the final collective is hidden behind PE work.

On-device layout is transposed ([feature, token]) except V (token-major, as
the ctx matmul's stationary operand). Softmax denominators come from a ones
column appended to V (l at PSUM partition 64); the two l rows are DMA-hopped
to partitions 0/32 of one tile, reciprocal'd in a single wide DVE op, and
broadcast across partitions with contract-1 PE matmuls. bk is dropped
entirely (softmax is invariant to a per-query logit offset); bv is folded
into the output bias (bo + Wo@bv); the 1/8 score scale into Wq/bq.

bf16 data path with fp32 PSUM accumulation. The ctx matmul for kv block n is
emitted after the scores for block n+1 so the PE never waits on the exp; Q/K/V
projections for batch b+1 are interleaved into batch b's attention as filler.
"""

import numpy as np
import ml_dtypes

B, T, C, NH, D = 4, 2048, 1024, 16, 64
P = 128
KC = C // P          # 8 contraction chunks
CH = 512             # q-chunk / projection token-chunk size
NCHUNK = T // CH     # 4 chunks per batch
TS = 256             # per-core token slice of each batch (T/8)

_CACHE = {}


def _build():
    import concourse.bacc as bacc
    import concourse.tile as tile
    import concourse.mybir as mybir
    from concourse.bass import ts, ds

    f32 = mybir.dt.float32
    bf16 = mybir.dt.bfloat16
    EXP = mybir.ActivationFunctionType.Exp
    MUL = mybir.AluOpType.mult
    ADD = mybir.AluOpType.add

    nc = bacc.Bacc("TRN2", target_bir_lowering=False, debug=False, num_devices=8)

    def din(name, shape, dt=bf16):
        return nc.dram_tensor(name, list(shape), dt, kind="ExternalInput").ap()

    xT = din("xT", (C, B * T))          # x^T, all batches (token = b*T + t)
    wq = din("wq", (C, P))              # (Wq/8)^T columns for this core's heads
    wk = din("wk", (C, P))
    wv = din("wv", (C, P))
    wo = din("wo", (C, C))              # full Wo^T
    bq = din("bq", (P, 1), f32)         # bq/8 slice
    bo = din("bo", (P, KC), f32)        # (bo + Wo@bv) chunked [128, 8]
    mask = din("mask", (P, P))          # 128x128 tril strip {0,1}
    out = nc.dram_tensor("out", [C, B * TS], f32, kind="ExternalOutput").ap()

    xT_v = xT.rearrange("(k p) t -> p k t", p=P)
    wq_v = wq.rearrange("(k p) m -> p k m", p=P)
    wk_v = wk.rearrange("(k p) m -> p k m", p=P)
    wv_v = wv.rearrange("(k p) m -> p k m", p=P)
    wo_v = wo.rearrange("(k p) m -> p k m", p=P)
    out_v = out.rearrange("(k p) t -> p k t", p=P)

    NBLK = T // P      # 16 kv blocks of 128 per batch

    from contextlib import ExitStack
    with ExitStack() as ctx:
        tc = ctx.enter_context(tile.TileContext(nc))

        consts = ctx.enter_context(tc.tile_pool(name="consts", bufs=1))
        wpool = ctx.enter_context(tc.tile_pool(name="w", bufs=1))
        xpool = ctx.enter_context(tc.tile_pool(name="x", bufs=3))
        qkpool = ctx.enter_context(tc.tile_pool(name="qk", bufs=2))
        vpool = ctx.enter_context(tc.tile_pool(name="v", bufs=2))
        ptpool = ctx.enter_context(tc.tile_pool(name="pt", bufs=4))
        lpool = ctx.enter_context(tc.tile_pool(name="l", bufs=2))
        cspool = ctx.enter_context(tc.tile_pool(name="cs", bufs=4))
        gpool = ctx.enter_context(tc.tile_pool(name="g", bufs=4))
        opool = ctx.enter_context(tc.tile_pool(name="o", bufs=2))
        psum = ctx.enter_context(tc.tile_pool(name="psum", bufs=2, space="PSUM"))
        dram = ctx.enter_context(tc.tile_pool(name="dram", bufs=2, space="DRAM"))

        # ---- constants ----
        from concourse.masks import make_identity
        bq_sb = consts.tile([P, 1], f32)
        bo_sb = consts.tile([P, KC], f32)
        msk_sb = consts.tile([P, P], bf16)
        ident = consts.tile([P, P], bf16)
        make_identity(nc, ident[:])
        nc.sync.dma_start(bq_sb[:], bq)
        nc.sync.dma_start(bo_sb[:], bo)
        nc.sync.dma_start(msk_sb[:], mask)
        # broadcast stationaries [1,...,1,0] living at partitions 0 and 32
        # (pairing with the hopped 1/l rows)
        pvec = consts.tile([33, D + 1], bf16)
        nc.vector.memset(pvec[0:1, 0:D], 1.0)
        nc.vector.memset(pvec[0:1, D : D + 1], 0.0)
        nc.vector.memset(pvec[32:33, 0:D], 1.0)
        nc.vector.memset(pvec[32:33, D : D + 1], 0.0)

        wq_sb = wpool.tile([P, KC, P], bf16, tag="wq", name="wq_sb")
        wk_sb = wpool.tile([P, KC, P], bf16, tag="wk", name="wk_sb")
        wv_sb = wpool.tile([P, KC, P], bf16, tag="wv", name="wv_sb")
        wo_sb = wpool.tile([P, KC, C], bf16, tag="wo", name="wo_sb")
        nc.sync.dma_start(wq_sb[:], wq_v)
        nc.sync.dma_start(wk_sb[:], wk_v)
        nc.sync.dma_start(wv_sb[:], wv_v)

        # ---- per-batch state ----
        QT = {}   # b -> [128, T] bf16 (2 heads x 64d on partitions)
        KT = {}
        VS = {}   # b -> [128, NBLK, 2, 65] token-major V (+ones col at 64)
        GC = {}   # b -> [128, KC, TS] gathered full-feature ctx
        BIN = {}  # b -> DRAM alltoall input bounce
        BOUT = {}
        XTT = {}

        def proj_chunk_q(b, t):
            """Tokens [CH*t, CH*t+CH) of batch b -> QT chunk (+x tile DMA)."""
            if b not in QT:
                QT[b] = qkpool.tile([P, T], bf16, tag="qt", name=f"qt{b}")
                KT[b] = qkpool.tile([P, T], bf16, tag="kt", name=f"kt{b}")
                VS[b] = vpool.tile([P, NBLK, 2, D + 1], bf16, tag="v",
                                   name=f"v{b}")
                nc.vector.memset(VS[b][:, :, :, D : D + 1], 1.0)
            xt = xpool.tile([P, KC, CH], bf16, tag="x", name=f"x{b}{t}")
            nc.sync.dma_start(xt[:], xT_v[:, :, ds(T * b + CH * t, CH)])
            XTT[(b, t)] = xt
            ps = psum.tile([P, CH], f32, tag="mix", name=f"pq{b}{t}")
            for k in range(KC):
                nc.tensor.matmul(ps[:], wq_sb[:, k, :], xt[:, k, :],
                                 start=(k == 0), stop=(k == KC - 1))
            nc.vector.tensor_tensor(QT[b][:, ds(CH * t, CH)], ps[:],
                                    bq_sb.to_broadcast((P, CH)), ADD)

        def proj_chunk_k(b, t):
            xt = XTT[(b, t)]
            ps = psum.tile([P, CH], f32, tag="mix", name=f"pk{b}{t}")
            for k in range(KC):
                nc.tensor.matmul(ps[:], wk_sb[:, k, :], xt[:, k, :],
                                 start=(k == 0), stop=(k == KC - 1))
            nc.vector.tensor_copy(KT[b][:, ds(CH * t, CH)], ps[:])

        VT = {}   # b -> [128, T] bf16 feature-major V^T (pre-transpose)

        def proj_chunk_v(b, t):
            """V^T chunk, feature-major (one 512-wide psum group like K)."""
            if b not in VT:
                VT[b] = vpool.tile([P, T], bf16, tag="vt", name=f"vt{b}")
            xt = XTT.pop((b, t))
            ps = psum.tile([P, CH], f32, tag="mix", name=f"pv{b}{t}")
            for k in range(KC):
                nc.tensor.matmul(ps[:], wv_sb[:, k, :], xt[:, k, :],
                                 start=(k == 0), stop=(k == KC - 1))
            nc.vector.tensor_copy(VT[b][:, ds(CH * t, CH)], ps[:])

        def v_transpose(b, t):
            """Turn V^T chunk into token-major VS blocks via PE transpose."""
            for i in range(4):
                blk = 4 * t + i
                ps = psum.tile([P, P], bf16, tag="mix", name=f"pt{b}{blk}")
                nc.tensor.transpose(ps[:], VT[b][:, ds(P * blk, P)], ident[:])
                nc.vector.tensor_copy(
                    VS[b][:, blk, :, 0:D],
                    ps.rearrange("t (h d) -> t h d", d=D))

        def oproj_unit(b, fc, nb=1):
            """Output features [128*fc, ...) for batches [b, b+nb) token slices.

            GC[b] must hold the gathered ctx for nb consecutive batches
            ([128, KC, nb*TS]); batches 1+2 share one tile so their O
            projection runs with a 512-wide moving dim."""
            g = GC[b]
            W = nb * TS
            ps = psum.tile([P, W], f32, tag="mix", name=f"po{b}{fc}")
            for k in range(KC):
                nc.tensor.matmul(ps[:], wo_sb[:, k, ts(fc, P)], g[:, k, 0:W],
                                 start=(k == 0), stop=(k == KC - 1))
            st = opool.tile([P, W], f32, tag="o", name=f"o{b}{fc}")
            nc.vector.tensor_tensor(st[:], ps[:],
                                    bo_sb[:, fc : fc + 1].to_broadcast((P, W)),
                                    ADD)
            nc.sync.dma_start(out_v[:, fc, ds(TS * b, W)], st[:])

        # ---- filler pump ----
        fillers = []
        fi = [0]

        def pump(n):
            k = 0
            while k < n and fi[0] < len(fillers):
                fillers[fi[0]]()
                fi[0] += 1
                k += 1

        def attn_chunk(b, qc):
            """Attention for q rows [CH*qc, CH*qc+CH), kv [0, CH*(qc+1)).

            Diagonal kv blocks (r = blk-4qc >= 0) compute only the W=512-128r
            rightmost columns; the 128-wide strip at the left of that window
            gets the tril mask. ctx matmuls lag scores by one block.
            """
            nkv = 4 * (qc + 1)
            ctx_ps = [psum.tile([D + 1, CH], f32, tag="ctx",
                                name=f"cx{b}{qc}{h}") for h in range(2)]
            prev = None   # (blk, pt, off, W)

            def emit_ctx(blk, pt, off, W):
                for hh in range(2):
                    nc.tensor.matmul(
                        ctx_ps[hh][:, ds(off, W)],
                        VS[b][:, blk, hh, :],
                        pt[:, hh, 0:W],
                        start=(blk == 0), stop=(blk == nkv - 1))

            for blk in range(nkv):
                r = blk - 4 * qc
                off = max(r, 0) * P
                W = CH - off
                st = psum.tile([P, 2, CH], f32, tag="st", name=f"st{b}{qc}{blk}")
                for hh in range(2):
                    nc.tensor.matmul(
                        st[:, hh, 0:W],
                        KT[b][ds(D * hh, D), ds(P * blk, P)],
                        QT[b][ds(D * hh, D), ds(CH * qc + off, W)],
                        start=True, stop=True)
                if prev is not None:
                    emit_ctx(*prev)
                    pump(1)
                pt = ptpool.tile([P, 2, CH], bf16, tag="pt",
                                 name=f"pt{b}{qc}{blk}")
                nc.scalar.activation(pt[:, :, 0:W], st[:, :, 0:W], EXP)
                if r >= 0:   # mask the 128-wide strip at the diagonal
                    nc.vector.tensor_tensor(
                        pt[:, :, 0:P], pt[:, :, 0:P],
                        msk_sb.rearrange("p (u q) -> p u q", u=1)
                              .to_broadcast((P, 2, P)), MUL)
                prev = (blk, pt, off, W)
            emit_ctx(*prev)
            # epilogue: ctx rows 0..63, l at row 64. Hop both l rows into one
            # tile (partitions 0 / 32), one wide reciprocal, then contract-1
            # broadcast matmuls.
            lrows = lpool.tile([33, CH], f32, tag="lr", name=f"lr{b}{qc}")
            nc.vector.tensor_copy(lrows[0:1, :], ctx_ps[0][D : D + 1, :])
            nc.vector.tensor_copy(lrows[32:33, :], ctx_ps[1][D : D + 1, :])
            linv = lpool.tile([33, CH], bf16, tag="li", name=f"li{b}{qc}")
            with nc.allow_low_precision(reason="1/l in bf16; ~0.2% rel err"):
                nc.vector.reciprocal(linv[:], lrows[:])
            pump(1)
            bc = [psum.tile([D + 1, CH], f32, tag="mix", name=f"bc{b}{qc}{h}")
                  for h in range(2)]
            for hh in range(2):
                nc.tensor.matmul(bc[hh][:], pvec[ds(32 * hh, 1), :],
                                 linv[ds(32 * hh, 1), :],
                                 start=True, stop=True)
            pump(1)
            for hh in range(2):
                bcs = cspool.tile([D, CH], bf16, tag="bcs",
                                  name=f"bcs{b}{qc}{hh}")
                nc.vector.tensor_copy(bcs[:], bc[hh][0:D, :])
                cs = cspool.tile([D, CH], bf16, tag="cs",
                                 name=f"cs{b}{qc}{hh}")
                nc.vector.tensor_tensor(cs[:], ctx_ps[hh][0:D, :],
                                        bcs[:], MUL)
                # ship straight into the alltoall input bounce
                nc.sync.dma_start(
                    BIN[b].rearrange("j p t -> p j t")
                         [ds(D * hh, D), ds(2 * qc, 2), :],
                    cs.rearrange("p (j t) -> p j t", t=TS))

        def alltoall(b):
            BOUT[b] = dram.tile([8, P, TS], bf16, tag="cout", name=f"co{b}")
            nc.gpsimd.collective_compute(
                "AllToAll", mybir.AluOpType.bypass,
                replica_groups=[list(range(8))],
                ins=[BIN[b].opt()], outs=[BOUT[b].opt()])
            # batches 1 and 2 share one gather tile so their O projection can
            # run 512 tokens wide; gathers ride the gpsimd queue behind the
            # collective (keeps the sync queue free for output DMAs)
            if b == 2:
                GC[2] = GC[1]
                half = 1
            else:
                GC[b] = gpool.tile([P, KC, (2 if b == 1 else 1) * TS], bf16,
                                   tag=("g2" if b == 1 else "g"),
                                   name=f"g{b}")
                half = 0
            nc.gpsimd.dma_start(GC[b][:, :, ds(half * TS, TS)],
                                BOUT[b].rearrange("s p t -> p s t"))

        # ---- emission schedule ----
        proj_chunk_q(0, 0); proj_chunk_k(0, 0)
        proj_chunk_v(0, 0); v_transpose(0, 0)
        nc.sync.dma_start(wo_sb[:], wo_v)
        for b in range(B):
            BIN[b] = dram.tile([8, P, TS], bf16, tag="cin", name=f"ci{b}")
            for qc in range(NCHUNK):
                fillers.clear(); fi[0] = 0
                if b == 0 and qc < 3:
                    fillers += [lambda t=qc + 1: proj_chunk_q(0, t),
                                lambda t=qc + 1: proj_chunk_k(0, t),
                                lambda t=qc + 1: proj_chunk_v(0, t),
                                lambda t=qc + 1: v_transpose(0, t)]
                if b < 3:
                    fillers += [lambda t=qc, bb=b + 1: proj_chunk_q(bb, t),
                                lambda t=qc, bb=b + 1: proj_chunk_k(bb, t),
                                lambda t=qc, bb=b + 1: proj_chunk_v(bb, t),
                                lambda t=qc, bb=b + 1: v_transpose(bb, t)]
                else:
                    # batch 0's output projection fills batch 3's attention
                    fillers += [lambda f=2 * qc + i: oproj_unit(0, f)
                                for i in range(2)]
                attn_chunk(b, qc)
                pump(len(fillers))   # flush leftovers
            alltoall(b)
        # tail: batches 1+2 output projection (512-wide) hides the last
        # collective; batch 3's follows once its gather lands
        for fc in range(KC):
            oproj_unit(1, fc, nb=2)
        for fc in range(KC):
            oproj_unit(3, fc)

    nc.compile()
    return nc


def _make_in_maps(x, Wq, bq, Wk, bk, Wv, bv, Wo, bo):
    bf = ml_dtypes.bfloat16
    xT = np.ascontiguousarray(
        x.transpose(2, 0, 1).reshape(C, B * T)).astype(bf)
    WqT8 = (Wq.T / 8.0).astype(bf)
    WkT = Wk.T.astype(bf)
    WvT = Wv.T.astype(bf)
    WoT = np.ascontiguousarray(Wo.T.astype(bf))
    bq8 = (bq / 8.0).astype(np.float32)
    bo_f = (bo + Wo @ bv).astype(np.float32)
    bo8 = np.ascontiguousarray(bo_f.reshape(KC, P).T)
    kv = np.arange(P)[:, None]
    q = np.arange(P)[None, :]
    msk = np.ascontiguousarray((kv <= q).astype(bf))
    in_maps = []
    for c in range(8):
        sl = slice(P * c, P * (c + 1))
        in_maps.append({
            "xT": xT,
            "wq": np.ascontiguousarray(WqT8[:, sl]),
            "wk": np.ascontiguousarray(WkT[:, sl]),
            "wv": np.ascontiguousarray(WvT[:, sl]),
            "wo": WoT,
            "bq": np.ascontiguousarray(bq8[sl, None]),
            "bo": bo8,
            "mask": msk,
        })
    return in_maps


def kernel(x, Wq, bq, Wk, bk, Wv, bv, Wo, bo):
    from concourse.bass_utils import run_bass_kernel_spmd

    x = np.asarray(x, np.float32)
    Wq = np.asarray(Wq, np.float32); bq = np.asarray(bq, np.float32)
    Wk = np.asarray(Wk, np.float32); bk = np.asarray(bk, np.float32)
    Wv = np.asarray(Wv, np.float32); bv = np.asarray(bv, np.float32)
    Wo = np.asarray(Wo, np.float32); bo = np.asarray(bo, np.float32)

    if "nc" not in _CACHE:
        _CACHE["nc"] = _build()
    nc = _CACHE["nc"]

    in_maps = _make_in_maps(x, Wq, bq, Wk, bk, Wv, bv, Wo, bo)
    res = run_bass_kernel_spmd(nc, in_maps, core_ids=list(range(8)))
    outf = np.empty((B, T, C), np.float32)
    for c in range(8):
        o = res.results[c]["out"]            # (C, B*TS) transposed
        for b in range(B):
            outf[b, TS * c : TS * (c + 1), :] = o[:, TS * b : TS * (b + 1)].T
    return outf


# revision 24
# speedup vs baseline: 1.0643x; 1.0643x over previous
"""Causal multi-head attention (B=4, T=2048, C=1024, 16 heads) on 8 TRN2 cores.

Tensor-parallel over heads: core c owns heads 2c, 2c+1 (128 features) and
computes Q/K/V projections + attention for those heads over ALL 4 batches.
Causal work uses exact extents (q-chunk 512, kv extent 512*(qc+1)); for the
four diagonal kv blocks only the columns right of the diagonal are computed
(W = 512-128r), so just one 128x128 tril strip needs masking. Every core runs
the same instruction stream (SPMD) - only weight slices differ.

After attention, a per-batch 512KB AllToAll redistributes context from
feature-sharded to token-sharded; each core then runs the output projection
for its 256-token slice of each batch (contract over the full 1024 features).
Output projections are deferred (batch 0's into batch 3's attention, batches
1-2's to the tail) so the final collective is hidden behind PE work.

On-device layout is transposed ([feature, token]) except V (token-major, as
the ctx matmul's stationary operand). Softmax denominators come from a ones
column appended to V (l at PSUM partition 64); the two l rows are DMA-hopped
to partitions 0/32 of one tile, reciprocal'd in a single wide DVE op, and
broadcast across partitions with contract-1 PE matmuls. bk is dropped
entirely (softmax is invariant to a per-query logit offset); bv is folded
into the output bias (bo + Wo@bv); the 1/8 score scale into Wq/bq.

bf16 data path with fp32 PSUM accumulation. The ctx matmul for kv block n is
emitted after the scores for block n+1 so the PE never waits on the exp; Q/K/V
projections for batch b+1 are interleaved into batch b's attention as filler.
"""

import numpy as np
import ml_dtypes

B, T, C, NH, D = 4, 2048, 1024, 16, 64
P = 128
KC = C // P          # 8 contraction chunks
CH = 512             # q-chunk / projection token-chunk size
NCHUNK = T // CH     # 4 chunks per batch
TS = 256             # per-core token slice of each batch (T/8)

_CACHE = {}


def _build():
    import concourse.bacc as bacc
    import concourse.tile as tile
    import concourse.mybir as mybir
    from concourse.bass import ts, ds

    f32 = mybir.dt.float32
    bf16 = mybir.dt.bfloat16
    EXP = mybir.ActivationFunctionType.Exp
    MUL = mybir.AluOpType.mult
    ADD = mybir.AluOpType.add

    nc = bacc.Bacc("TRN2", target_bir_lowering=False, debug=False, num_devices=8)

    def din(name, shape, dt=bf16):
        return nc.dram_tensor(name, list(shape), dt, kind="ExternalInput").ap()

    xT = din("xT", (C, B * T))          # x^T, all batches (token = b*T + t)
    wq = din("wq", (C, P))              # (Wq/8)^T columns for this core's heads
    wk = din("wk", (C, P))
    wv = din("wv", (C, P))
    wo = din("wo", (C, C))              # full Wo^T
    bq = din("bq", (P, 1), f32)         # bq/8 slice
    bo = din("bo", (P, KC), f32)        # (bo + Wo@bv) chunked [128, 8]
    mask = din("mask", (P, P))          # 128x128 tril strip {0,1}
    out = nc.dram_tensor("out", [C, B * TS], f32, kind="ExternalOutput").ap()

    xT_v = xT.rearrange("(k p) t -> p k t", p=P)
    wq_v = wq.rearrange("(k p) m -> p k m", p=P)
    wk_v = wk.rearrange("(k p) m -> p k m", p=P)
    wv_v = wv.rearrange("(k p) m -> p k m", p=P)
    wo_v = wo.rearrange("(k p) m -> p k m", p=P)
    out_v = out.rearrange("(k p) t -> p k t", p=P)

    NBLK = T // P      # 16 kv blocks of 128 per batch

    from contextlib import ExitStack
    with ExitStack() as ctx:
        tc = ctx.enter_context(tile.TileContext(nc))

        consts = ctx.enter_context(tc.tile_pool(name="consts", bufs=1))
        wpool = ctx.enter_context(tc.tile_pool(name="w", bufs=1))
        xpool = ctx.enter_context(tc.tile_pool(name="x", bufs=3))
        qkpool = ctx.enter_context(tc.tile_pool(name="qk", bufs=2))
        vpool = ctx.enter_context(tc.tile_pool(name="v", bufs=2))
        ptpool = ctx.enter_context(tc.tile_pool(name="pt", bufs=4))
        lpool = ctx.enter_context(tc.tile_pool(name="l", bufs=2))
        cspool = ctx.enter_context(tc.tile_pool(name="cs", bufs=4))
        gpool = ctx.enter_context(tc.tile_pool(name="g", bufs=4))
        opool = ctx.enter_context(tc.tile_pool(name="o", bufs=2))
        psum = ctx.enter_context(tc.tile_pool(name="psum", bufs=2, space="PSUM"))
        dram = ctx.enter_context(tc.tile_pool(name="dram", bufs=2, space="DRAM"))

        # ---- constants ----
        from concourse.masks import make_identity
        bq_sb = consts.tile([P, 1], f32)
        bo_sb = consts.tile([P, KC], f32)
        msk_sb = consts.tile([P, P], bf16)
        ident = consts.tile([P, P], bf16)
        make_identity(nc, ident[:])
        nc.sync.dma_start(bq_sb[:], bq)
        nc.sync.dma_start(bo_sb[:], bo)
        nc.sync.dma_start(msk_sb[:], mask)
        # broadcast stationaries [1,...,1,0] living at partitions 0 and 32
        # (pairing with the hopped 1/l rows)
        pvec = consts.tile([33, D + 1], bf16)
        nc.vector.memset(pvec[0:1, 0:D], 1.0)
        nc.vector.memset(pvec[0:1, D : D + 1], 0.0)
        nc.vector.memset(pvec[32:33, 0:D], 1.0)
        nc.vector.memset(pvec[32:33, D : D + 1], 0.0)

        wq_sb = wpool.tile([P, KC, P], bf16, tag="wq", name="wq_sb")
        wk_sb = wpool.tile([P, KC, P], bf16, tag="wk", name="wk_sb")
        wv_sb = wpool.tile([P, KC, P], bf16, tag="wv", name="wv_sb")
        wo_sb = wpool.tile([P, KC, C], bf16, tag="wo", name="wo_sb")
        nc.sync.dma_start(wq_sb[:], wq_v)

        # ---- per-batch state ----
        QT = {}   # b -> [128, T] bf16 (2 heads x 64d on partitions)
        KT = {}
        VS = {}   # b -> [128, NBLK, 2, 65] token-major V (+ones col at 64)
        GC = {}   # b -> [128, KC, TS] gathered full-feature ctx
        BIN = {}  # b -> DRAM alltoall input bounce
        BOUT = {}
        XTT = {}

        def proj_chunk_q(b, t):
            """Tokens [CH*t, CH*t+CH) of batch b -> QT chunk (+x tile DMA)."""
            if b not in QT:
                QT[b] = qkpool.tile([P, T], bf16, tag="qt", name=f"qt{b}")
                KT[b] = qkpool.tile([P, T], bf16, tag="kt", name=f"kt{b}")
                VS[b] = vpool.tile([P, NBLK, 2, D + 1], bf16, tag="v",
                                   name=f"v{b}")
                nc.vector.memset(VS[b][:, :, :, D : D + 1], 1.0)
            xt = xpool.tile([P, KC, CH], bf16, tag="x", name=f"x{b}{t}")
            nc.sync.dma_start(xt[:], xT_v[:, :, ds(T * b + CH * t, CH)])
            XTT[(b, t)] = xt
            ps = psum.tile([P, CH], f32, tag="mix", name=f"pq{b}{t}")
            for k in range(KC):
                nc.tensor.matmul(ps[:], wq_sb[:, k, :], xt[:, k, :],
                                 start=(k == 0), stop=(k == KC - 1))
            nc.vector.tensor_tensor(QT[b][:, ds(CH * t, CH)], ps[:],
                                    bq_sb.to_broadcast((P, CH)), ADD)

        def proj_chunk_k(b, t):
            xt = XTT[(b, t)]
            ps = psum.tile([P, CH], f32, tag="mix", name=f"pk{b}{t}")
            for k in range(KC):
                nc.tensor.matmul(ps[:], wk_sb[:, k, :], xt[:, k, :],
                                 start=(k == 0), stop=(k == KC - 1))
            nc.vector.tensor_copy(KT[b][:, ds(CH * t, CH)], ps[:])

        VT = {}   # b -> [128, T] bf16 feature-major V^T (pre-transpose)

        def proj_chunk_v(b, t):
            """V^T chunk, feature-major (one 512-wide psum group like K)."""
            if b not in VT:
                VT[b] = vpool.tile([P, T], bf16, tag="vt", name=f"vt{b}")
            xt = XTT.pop((b, t))
            ps = psum.tile([P, CH], f32, tag="mix", name=f"pv{b}{t}")
            for k in range(KC):
                nc.tensor.matmul(ps[:], wv_sb[:, k, :], xt[:, k, :],
                                 start=(k == 0), stop=(k == KC - 1))
            nc.vector.tensor_copy(VT[b][:, ds(CH * t, CH)], ps[:])

        def v_transpose(b, t):
            """Turn V^T chunk into token-major VS blocks via PE transpose."""
            for i in range(4):
                blk = 4 * t + i
                ps = psum.tile([P, P], bf16, tag="mix", name=f"pt{b}{blk}")
                nc.tensor.transpose(ps[:], VT[b][:, ds(P * blk, P)], ident[:])
                nc.vector.tensor_copy(
                    VS[b][:, blk, :, 0:D],
                    ps.rearrange("t (h d) -> t h d", d=D))

        def oproj_unit(b, fc, nb=1):
            """Output features [128*fc, ...) for batches [b, b+nb) token slices.

            GC[b] must hold the gathered ctx for nb consecutive batches
            ([128, KC, nb*TS]); batches 1+2 share one tile so their O
            projection runs with a 512-wide moving dim."""
            g = GC[b]
            W = nb * TS
            ps = psum.tile([P, W], f32, tag="mix", name=f"po{b}{fc}")
            for k in range(KC):
                nc.tensor.matmul(ps[:], wo_sb[:, k, ts(fc, P)], g[:, k, 0:W],
                                 start=(k == 0), stop=(k == KC - 1))
            st = opool.tile([P, W], f32, tag="o", name=f"o{b}{fc}")
            nc.vector.tensor_tensor(st[:], ps[:],
                                    bo_sb[:, fc : fc + 1].to_broadcast((P, W)),
                                    ADD)
            nc.sync.dma_start(out_v[:, fc, ds(TS * b, W)], st[:])

        # ---- filler pump ----
        fillers = []
        fi = [0]

        def pump(n):
            k = 0
            while k < n and fi[0] < len(fillers):
                fillers[fi[0]]()
                fi[0] += 1
                k += 1

        def attn_chunk(b, qc):
            """Attention for q rows [CH*qc, CH*qc+CH), kv [0, CH*(qc+1)).

            Diagonal kv blocks (r = blk-4qc >= 0) compute only the W=512-128r
            rightmost columns; the 128-wide strip at the left of that window
            gets the tril mask. ctx matmuls lag scores by one block.
            """
            nkv = 4 * (qc + 1)
            ctx_ps = [psum.tile([D + 1, CH], f32, tag="ctx", bufs=2,
                                name=f"cx{b}{qc}{h}") for h in range(2)]
            prev = None   # (blk, pt, off, w)

            def emit_ctx(blk, pt, off, w):
                for hh in range(2):
                    nc.tensor.matmul(
                        ctx_ps[hh][:, ds(off, w)],
                        VS[b][:, blk, hh, :],
                        pt[:, hh, 0:w],
                        start=(blk == 0), stop=(blk == nkv - 1),
                        skip_group_check=True)

            for blk in range(nkv):
                r = blk - 4 * qc
                off = max(r, 0) * P
                w = CH - off
                st = psum.tile([P, 2, CH], f32, tag="st",
                               name=f"st{b}{qc}{blk}")
                for hh in range(2):
                    nc.tensor.matmul(
                        st[:, hh, 0:w],
                        KT[b][ds(D * hh, D), ds(P * blk, P)],
                        QT[b][ds(D * hh, D), ds(CH * qc + off, w)],
                        start=True, stop=True)
                if prev is not None:
                    emit_ctx(*prev)
                    pump(1)
                pt = ptpool.tile([P, 2, CH], bf16, tag="pt",
                                 name=f"pt{b}{qc}{blk}")
                nc.scalar.activation(pt[:, :, 0:w], st[:, :, 0:w], EXP)
                if r >= 0:   # mask the strip at the diagonal
                    nc.vector.tensor_tensor(
                        pt[:, :, 0:P], pt[:, :, 0:P],
                        msk_sb.rearrange("p (u q) -> p u q", u=1)
                              .to_broadcast((P, 2, P)), MUL)
                prev = (blk, pt, off, w)
            emit_ctx(*prev)
            # epilogue: ctx rows 0..63, l at row 64. Hop both l rows into one
            # tile (partitions 0 / 32), one wide reciprocal, then contract-1
            # broadcast matmuls.
            lrows = lpool.tile([33, CH], f32, tag="lr", name=f"lr{b}{qc}")
            nc.vector.tensor_copy(lrows[0:1, :], ctx_ps[0][D : D + 1, :])
            nc.vector.tensor_copy(lrows[32:33, :], ctx_ps[1][D : D + 1, :])
            linv = lpool.tile([33, CH], bf16, tag="li", name=f"li{b}{qc}")
            with nc.allow_low_precision(reason="1/l in bf16; ~0.2% rel err"):
                nc.vector.reciprocal(linv[:], lrows[:])
            pump(2)
            bc = [psum.tile([D + 1, CH], f32, tag="mix", name=f"bc{b}{qc}{h}")
                  for h in range(2)]
            for hh in range(2):
                nc.tensor.matmul(bc[hh][:], pvec[ds(32 * hh, 1), :],
                                 linv[ds(32 * hh, 1), :],
                                 start=True, stop=True)
            pump(1)
            for hh in range(2):
                bcs = cspool.tile([D, CH], bf16, tag="bcs",
                                  name=f"bcs{b}{qc}{hh}")
                nc.vector.tensor_copy(bcs[:], bc[hh][0:D, :])
                cs = cspool.tile([D, CH], bf16, tag="cs",
                                 name=f"cs{b}{qc}{hh}")
                nc.vector.tensor_tensor(cs[:], ctx_ps[hh][0:D, :],
                                        bcs[:], MUL)
                # ship straight into the alltoall input bounce
                nc.sync.dma_start(
                    BIN[b].rearrange("j p t -> p j t")
                         [ds(D * hh, D), ds(2 * qc, 2), :],
                    cs.rearrange("p (j t) -> p j t", t=TS))

        def alltoall(b):
            BOUT[b] = dram.tile([8, P, TS], bf16, tag="cout", name=f"co{b}")
            nc.gpsimd.collective_compute(
                "AllToAll", mybir.AluOpType.bypass,
                replica_groups=[list(range(8))],
                ins=[BIN[b].opt()], outs=[BOUT[b].opt()])
            # batches 1 and 2 share one gather tile so their O projection can
            # run 512 tokens wide; gathers ride the gpsimd queue behind the
            # collective (keeps the sync queue free for output DMAs)
            if b == 2:
                GC[2] = GC[1]
                half = 1
            else:
                GC[b] = gpool.tile([P, KC, (2 if b == 1 else 1) * TS], bf16,
                                   tag=("g2" if b == 1 else "g"),
                                   name=f"g{b}")
                half = 0
            nc.gpsimd.dma_start(GC[b][:, :, ds(half * TS, TS)],
                                BOUT[b].rearrange("s p t -> p s t"))

        # ---- emission schedule ----
        proj_chunk_q(0, 0)
        nc.sync.dma_start(wk_sb[:], wk_v)
        nc.sync.dma_start(wv_sb[:], wv_v)
        proj_chunk_k(0, 0)
        proj_chunk_v(0, 0); v_transpose(0, 0)
        nc.sync.dma_start(wo_sb[:], wo_v)
        for b in range(B):
            BIN[b] = dram.tile([8, P, TS], bf16, tag="cin", name=f"ci{b}")
            for qc in range(NCHUNK):
                fillers.clear(); fi[0] = 0
                if b == 0 and qc < 3:
                    fillers += [lambda t=qc + 1: proj_chunk_q(0, t),
                                lambda t=qc + 1: proj_chunk_k(0, t),
                                lambda t=qc + 1: proj_chunk_v(0, t),
                                lambda t=qc + 1: v_transpose(0, t)]
                if b < 3:
                    fillers += [lambda t=qc, bb=b + 1: proj_chunk_q(bb, t),
                                lambda t=qc, bb=b + 1: proj_chunk_k(bb, t),
                                lambda t=qc, bb=b + 1: proj_chunk_v(bb, t),
                                lambda t=qc, bb=b + 1: v_transpose(bb, t)]
                else:
                    # batch 0's output projection fills batch 3's attention
                    fillers += [lambda f=2 * qc + i: oproj_unit(0, f)
                                for i in range(2)]
                attn_chunk(b, qc)
                pump(len(fillers))   # flush leftovers
            alltoall(b)
        # tail: batches 1+2 output projection (512-wide) hides the last
        # collective; batch 3's follows once its gather lands
        for fc in range(KC):
            oproj_unit(1, fc, nb=2)
        for fc in range(KC):
            oproj_unit(3, fc)

    nc.compile()
    return nc


def _make_in_maps(x, Wq, bq, Wk, bk, Wv, bv, Wo, bo):
    bf = ml_dtypes.bfloat16
    xT = np.ascontiguousarray(
        x.transpose(2, 0, 1).reshape(C, B * T)).astype(bf)
    WqT8 = (Wq.T / 8.0).astype(bf)
    WkT = Wk.T.astype(bf)
    WvT = Wv.T.astype(bf)
    WoT = np.ascontiguousarray(Wo.T.astype(bf))
    bq8 = (bq / 8.0).astype(np.float32)
    bo_f = (bo + Wo @ bv).astype(np.float32)
    bo8 = np.ascontiguousarray(bo_f.reshape(KC, P).T)
    kv = np.arange(P)[:, None]
    q = np.arange(P)[None, :]
    msk = np.ascontiguousarray((kv <= q).astype(bf))
    in_maps = []
    for c in range(8):
        sl = slice(P * c, P * (c + 1))
        in_maps.append({
            "xT": xT,
            "wq": np.ascontiguousarray(WqT8[:, sl]),
            "wk": np.ascontiguousarray(WkT[:, sl]),
            "wv": np.ascontiguousarray(WvT[:, sl]),
            "wo": WoT,
            "bq": np.ascontiguousarray(bq8[sl, None]),
            "bo": bo8,
            "mask": msk,
        })
    return in_maps


def kernel(x, Wq, bq, Wk, bk, Wv, bv, Wo, bo):
    from concourse.bass_utils import run_bass_kernel_spmd

    x = np.asarray(x, np.float32)
    Wq = np.asarray(Wq, np.float32); bq = np.asarray(bq, np.float32)
    Wk = np.asarray(Wk, np.float32); bk = np.asarray(bk, np.float32)
    Wv = np.asarray(Wv, np.float32); bv = np.asarray(bv, np.float32)
    Wo = np.asarray(Wo, np.float32); bo = np.asarray(bo, np.float32)

    if "nc" not in _CACHE:
        _CACHE["nc"] = _build()
    nc = _CACHE["nc"]

    in_maps = _make_in_maps(x, Wq, bq, Wk, bk, Wv, bv, Wo, bo)
    res = run_bass_kernel_spmd(nc, in_maps, core_ids=list(range(8)))
    outf = np.empty((B, T, C), np.float32)
    for c in range(8):
        o = res.results[c]["out"]            # (C, B*TS) transposed
        for b in range(B):
            outf[b, TS * c : TS * (c + 1), :] = o[:, TS * b : TS * (b + 1)].T
    return outf
